# revision 34
# baseline (speedup 1.0000x reference)
"""Trainium2 Bass kernel for one transformer Block (causal attn + SwiGLU MLP).

Problem: x (2048, 768), H=12 heads, causal self-attention + SwiGLU MLP,
fp32 I/O. 8 NeuronCores.

Sharding: sequence-sharded, no collectives: core i owns rows
256*i..256*(i+1); ln1/K/V are recomputed over the full sequence on every
core; weights replicated.

v2 speed structure (HW-measured v1 = 379us, baseline = 503us):
  - fp8e4 (TRN e4m3, max 240) DoubleRow matmuls (2x contraction/cycle)
    for qkv projections, AV (probs x V), Wo, f@Wsw, f@Vsw, g@Wproj.
    Power-of-2 scales keep every fp8 cast far below the 240 saturation
    point (TRN saturates to Inf, not clip).  Host-sim rel err 1.49e-2
    (gate 2e-2); HW tracked sim within 2e-5 on v1.  Wfc + QK stay bf16
    (fc-fp8 alone costs +1.2e-2).
  - Attention: exp on scalar engine straight out of PSUM with ln(32)
    folded into the bias (probs carry x32 for the fp8 cast), causal mask
    applied multiplicatively AFTER exp on gpsimd (scores bounded ~2, no
    overflow), kv-tiles in batches of 2 so AV runs as one DoubleRow
    matmul per head over the pair.  PSUM: 2x2 banks scores (two heads
    share a bank via a start/stop accumulation pair - both matmuls MUST
    be at PE array row 0: mixed-row groups hang the PE, hence the
    kT_odd/qT_odd partition-0:64 copies), 1 bank y, 1 denominator bcast.
  - Weight residency: Wsw pass-0 (4.7MB fp8) is DMA'd into SBUF during
    attention (the DMA-idle window in v1); the remaining 3 passes stream
    through a 4-buffer pool fed round-robin from 3 engine queues.
  - PSUM->SBUF readout work split across scalar AND vector so neither
    engine gates the LN/transpose pipeline in prep.
"""

from contextlib import ExitStack
import math
import os

import numpy as np
import ml_dtypes

import concourse.bass as bass
import concourse.mybir as mybir
import concourse.tile as tile
from concourse import bacc, bass_utils
from concourse.masks import make_identity

AF = mybir.ActivationFunctionType
PM = mybir.MatmulPerfMode
BF16 = mybir.dt.bfloat16
F32 = mybir.dt.float32
FP8 = mybir.dt.float8e4

T, C, H, D = 2048, 768, 12, 64
NCORES = 8
R = T // NCORES            # 256 rows per core
C4 = 4 * C                 # 3072
EPS = 1e-5
NT = R // 128              # 2   row tiles per core
NCT = C // 128             # 6   channel tiles
NJT = C4 // 128            # 24  hidden tiles
NKV = T // 128             # 16  kv tiles

# fp8 power-of-2 scales (|val*scale| << 240 always; saturation -> Inf!)
S_H = 8.0                  # ln1/ln2 output (|h| <= sqrt(C) ~ 27.7 -> 221)
S_W = 512.0                # weights (|W| ~ 0.1 max -> ~56)
S_F = 16.0                 # f (|f| ~ 6 max -> ~96)
S_P = 32.0                 # probs (exp <= ~6.2 -> ~198) and v (~3.5 -> 112)
S_G = 32.0                 # g (|g| max 5.1 -> 163; fp8 saturates to Inf past 240!)
DQ_KV = 1.0 / (S_H * S_W)          # kT dequant
DQ_V = S_P / (S_H * S_W)           # v -> fp8 carrying 32*v
DQ_Q = 1.0 / (S_H * S_W * 8.0)     # q dequant, 1/sqrt(D) folded
LN32 = math.log(S_P)               # folded into exp bias: probs carry x32
DQ_WO = 1.0 / (S_P * S_W)          # Wo psum dequant (y carried 32x)
S_ACC = S_F * S_W                  # f@Wsw psum carries 8192*z
DQ_GT = S_G / (S_ACC * S_ACC)      # g(bf16, 8192^2) -> fp8 64*g
DQ_P = 1.0 / (S_G * S_W)           # proj psum dequant

PHASE = os.environ.get("KPHASE", "full")  # debug bisect: prep|attn|full
NODR = set(os.environ.get("KNODR", "").split(","))  # disable DoubleRow: av,wo,proj


def _ln_stats(nc, pool, in_ap, eps_sb):
    """Per-partition (mean, rs); rs = 8/sqrt(var+eps) (fp8 scale folded)."""
    stats = pool.tile([128, 3, 6], F32, name="ln_stats", tag="ln_stats", bufs=2)
    for sg in range(3):
        nc.vector.bn_stats(stats[:, sg, :], in_ap[:, sg * 256:(sg + 1) * 256])
    mv = pool.tile([128, 2], F32, name="ln_mv", tag="ln_mv", bufs=2)
    nc.vector.bn_aggr(mv, stats)
    sd = pool.tile([128, 1], F32, name="ln_sd", tag="ln_sd", bufs=2)
    nc.scalar.activation(sd, mv[:, 1:2], AF.Sqrt, bias=eps_sb, scale=1.0 / 64)
    rs = pool.tile([128, 1], F32, name="ln_rs", tag="ln_rs", bufs=2)
    nc.vector.reciprocal(rs, sd)
    return mv, rs


def _body(tc, io):
    ctx = ExitStack()
    try:
        _body_inner(tc, io, ctx)
    finally:
        ctx.close()


def _body_inner(tc, io, ctx):
    nc = tc.nc
    ts = bass.ts

    persist = ctx.enter_context(tc.tile_pool(name="persist", bufs=1))
    lnpool = ctx.enter_context(tc.tile_pool(name="lnpool", bufs=1))

    id128 = persist.tile([128, 128], BF16)
    make_identity(nc, id128)
    eps_sb = persist.tile([128, 1], F32)
    nc.vector.memset(eps_sb, EPS / 64.0)
    ones65 = persist.tile([65, 64], F32)
    nc.vector.memset(ones65[:], 0.0)
    nc.vector.memset(ones65[64:65, :], 1.0)
    ln32_sb = persist.tile([128, 1], F32)
    nc.vector.memset(ln32_sb, LN32)

    x_sb = persist.tile([128, NT, C], F32)
    nc.sync.dma_start(x_sb[:], io["xp"][:])
    x2_sb = persist.tile([128, NT, C], F32)

    # ---------------- prep phase: hT(fp8), kT, v, qT ----------------
    with (
        tc.tile_pool(name="awpool", bufs=1) as awpool,
        tc.tile_pool(name="wswpool", bufs=4) as wswpool,
    ):
        apx = ExitStack()
        apool = apx.enter_context(tc.tile_pool(name="apool", bufs=1))
        mask_sb = apool.tile([128, NKV, 2 * R], FP8)

        qT_sb = apool.tile([128, NCT, R], FP8)
        kT_res = apool.tile([128, NCT, T], FP8)
        v_res = apool.tile([128, NKV, 12, 68], FP8)
        nc.vector.memset(v_res[:, :, :, 64:65], 1.0)
        nc.vector.memset(v_res[:, :, :, 65:68], 0.0)
        # odd-head copies on partitions 0:64: a PE accumulation group must
        # not mix matmuls at different array-row offsets (row-64 + shared
        # bank hangs the PE), so the j=1 QK matmul reads row-0 copies.
        kT_odd = apool.tile([64, NCT, T], FP8)
        qT_odd = apool.tile([64, NCT, R], FP8)

        hpx = ExitStack()
        hTpool = hpx.enter_context(tc.tile_pool(name="hTpool", bufs=1))
        hT8_full = hTpool.tile([128, NCT, T], FP8)
        hT8_own = hTpool.tile([128, NCT, R], FP8)

        with (
            tc.tile_pool(name="hpool", bufs=3) as hpool,
            tc.tile_pool(name="wkvpool", bufs=1) as wkvpool,
            tc.tile_pool(name="tpsum", bufs=3, space="PSUM") as tpsum,
            tc.tile_pool(name="qpsum", bufs=2, space="PSUM") as qpsum,
        ):
            wq_sb = wkvpool.tile([128, NCT, C], FP8)
            nc.scalar.dma_start(wq_sb[:], io["wqp"][:])
            wk_sb = wkvpool.tile([128, NCT, C], FP8)
            nc.scalar.dma_start(wk_sb[:], io["wkp"][:])
            wv_sb = wkvpool.tile([128, NCT, C], FP8)
            nc.scalar.dma_start(wv_sb[:], io["wvp"][:])
            nc.gpsimd.dma_start(mask_sb[:], io["maskp"][:])

            def ln_transpose(src_ap, dst_ap, dst_off, xtra=0):
                """LN a 128-row tile; write its transpose (x8, fp8).
                PSUM->SBUF copies alternate vector/scalar to split load."""
                mv, rs = _ln_stats(nc, lnpool, src_ap, eps_sb)
                ht = hpool.tile([128, C], BF16, name="ht", tag="ht")
                nc.vector.tensor_scalar(
                    out=ht[:], in0=src_ap, scalar1=mv[:, 0:1], scalar2=rs,
                    op0=mybir.AluOpType.subtract, op1=mybir.AluOpType.mult)
                for ct in range(NCT):
                    pst = tpsum.tile([128, 128], BF16, name="pst", tag="pst")
                    nc.tensor.transpose(pst[:], ht[:, ts(ct, 128)], id128[:])
                    dst = dst_ap[:, ct, dst_off:dst_off + 128]
                    if (ct + xtra) % 2 == 0:
                        nc.vector.tensor_copy(dst, pst[:])
                    else:
                        nc.scalar.copy(dst, pst[:])

            # own rows first (q production + LN pipeline warmup)
            for tt in range(NT):
                ln_transpose(x_sb[:, tt, :], hT8_own, 128 * tt, tt)
            for dt in range(NCT):
                psq = qpsum.tile([128, R], F32, name="psq", tag="psk")
                for cp in range(3):
                    nc.tensor.matmul(psq[:], wq_sb[:, 2 * cp:2 * cp + 2,
                                                   ts(dt, 128)],
                                     hT8_own[:, 2 * cp:2 * cp + 2, :],
                                     start=(cp == 0), stop=(cp == 2),
                                     perf_mode=PM.DoubleRow)
                nc.scalar.activation(qT_sb[:, dt, :], psq[:], AF.Copy,
                                     scale=64.0 * DQ_Q)

            # full sequence in 512-col chunks: LN+transpose (vector) of
            # chunk c+1 overlaps kT/v matmuls (PE) of chunk c
            xts = {}
            xts[0] = hpool.tile([128, 4, C], BF16, name="xt4", tag="xt4",
                                bufs=2)
            nc.gpsimd.dma_start(xts[0][:], io["xfull"][:, 0:4, :])
            for ch in range(4):
                if ch + 1 < 4:
                    xts[ch + 1] = hpool.tile([128, 4, C], BF16, name="xt4",
                                             tag="xt4", bufs=2)
                    nc.gpsimd.dma_start(xts[ch + 1][:],
                                        io["xfull"][:, 4 * ch + 4:4 * ch + 8, :])
                xt4 = xts.pop(ch)
                for t4 in range(4):
                    tt = 4 * ch + t4
                    ln_transpose(xt4[:, t4, :], hT8_full, 128 * tt, t4)
                for dt in range(NCT):
                    psk = qpsum.tile([128, 512], F32, name="psk", tag="psk")
                    for cp in range(3):
                        nc.tensor.matmul(psk[:], wk_sb[:, 2 * cp:2 * cp + 2,
                                                       ts(dt, 128)],
                                         hT8_full[:, 2 * cp:2 * cp + 2,
                                                  ts(ch, 512)],
                                         start=(cp == 0), stop=(cp == 2),
                                         perf_mode=PM.DoubleRow)
                    nc.scalar.activation(kT_res[:, dt, ts(ch, 512)], psk[:],
                                         AF.Copy, scale=32.0 * DQ_KV)
                for t4 in range(4):
                    tt = 4 * ch + t4
                    for oh in range(2):
                        psv = qpsum.tile([128, 384], F32, name="psv",
                                         tag="psk")
                        for cp in range(3):
                            nc.tensor.matmul(
                                psv[:],
                                hT8_full[:, 2 * cp:2 * cp + 2, ts(tt, 128)],
                                wv_sb[:, 2 * cp:2 * cp + 2, ts(oh, 384)],
                                start=(cp == 0), stop=(cp == 2),
                                perf_mode=PM.DoubleRow)
                        nc.scalar.activation(
                            v_res[:, tt, 6 * oh:6 * oh + 6, 0:64], psv[:],
                            AF.Copy, scale=DQ_V)

        hpx.close()
        for dt in range(NCT):
            nc.gpsimd.dma_start(kT_odd[:, dt, :], kT_res[64:128, dt, :])
        nc.gpsimd.dma_start(qT_odd[:], qT_sb[64:128, :, :])

        if PHASE == "prep":
            dbg = persist.tile([128, NT, C], F32)
            nc.vector.tensor_copy(dbg[:, 0, :], kT_res[:, 0, 0:768])
            nc.vector.tensor_copy(dbg[:, 1, :], qT_sb[:, 0:3, :].rearrange(
                "p a b -> p (a b)"))
            nc.sync.dma_start(io["out"][:], dbg[:])
            apx.close()
            return

        # prefetch next-phase weights into the attention DMA-idle window
        wo_sb = apool.tile([64, NCT, 2, C], FP8)
        nc.scalar.dma_start(wo_sb[:], io["wop"][:])
        wfc_sb = awpool.tile([128, NCT, C4], BF16)
        nc.scalar.dma_start(wfc_sb[:], io["wfcp"][:])
        NPRE = 12  # wsw pass-0 chunks prefetched into SBUF
        wsw0_sb = awpool.tile([128, NPRE, 2, 1536], FP8)
        for jp in range(NPRE):
            nc.sync.dma_start(wsw0_sb[:, jp], io["wswp"][0, jp])

        # ---------------- attention ----------------
        yT8 = apool.tile([64, H, R], FP8)
        with (
            tc.tile_pool(name="apsum", bufs=2, space="PSUM") as apsum,
            tc.tile_pool(name="ypsum", bufs=1, space="PSUM") as ypsum,
            tc.tile_pool(name="bcpsum", bufs=1, space="PSUM") as bcpsum,
            tc.tile_pool(name="ampool", bufs=3) as ampool,
            tc.tile_pool(name="dnpool", bufs=4) as dnpool,
        ):
            for g in range(6):
                heads = [2 * g, 2 * g + 1]
                ct = g
                y_ps = ypsum.tile([68, 2, R], F32, name="y_ps", tag="y_ps")
                prev = None  # (axm tile, k0)
                for k0 in range(0, NKV, 2):
                    # balanced causal: kv tiles 8..15 only attend q-tile B
                    # (cols 128:256); kv 0..7 attend both q-tiles (N=256)
                    q0 = 128 if k0 >= 8 else 0
                    a_ps = apsum.tile([128, 2, 2, R], F32, name="a_ps",
                                      tag="a_ps")
                    for s in range(2):
                        kvt = k0 + s
                        for j in range(2):
                            # heads share one PSUM bank: j=0 starts the
                            # group (clears the 2KB zone), j=1 accumulates
                            # into its untouched half and stops.  Both at
                            # PE row 0 via the kT_odd/qT_odd copies.
                            kt = kT_res if j == 0 else kT_odd
                            qt = qT_sb if j == 0 else qT_odd
                            nc.tensor.matmul(
                                a_ps[:, s, j, 0:R - q0],
                                kt[0:64, ct, ts(kvt, 128)],
                                qt[0:64, ct, q0:R],
                                start=(j == 0), stop=(j == 1))
                    axe = ampool.tile([128, 2, 2, R], BF16, name="axe",
                                      tag="axe")
                    # exp(score + ln32): probs carry x32 for the fp8 cast
                    nc.scalar.activation(axe[:, :, :, 0:R - q0],
                                         a_ps[:, :, :, 0:R - q0], AF.Exp,
                                         bias=ln32_sb, scale=1.0 / 2048.0)
                    axm = ampool.tile([128, 2, 2, R], FP8, name="axm",
                                      tag="axm")
                    meng = nc.vector if (k0 // 2) % 2 == 0 else nc.gpsimd
                    meng.tensor_mul(
                        axm[:, :, :, 0:R - q0], axe[:, :, :, 0:R - q0],
                        mask_sb[:, k0:k0 + 2, :].rearrange(
                            "p k (a b) -> p k a b", a=2)[:, :, :, q0:R])
                    def av_pair(pm_, pk0, last):
                        pq0 = 128 if pk0 >= 8 else 0
                        for j, hh in enumerate(heads):
                            st = (pk0 == 0 and j == 0)
                            sp = last and (j == 1)
                            if "av" in NODR:
                                for s_ in range(2):
                                    nc.tensor.matmul(
                                        y_ps[:, j, pq0:R],
                                        v_res[:, pk0 + s_, hh, :],
                                        pm_[:, s_, j, 0:R - pq0],
                                        start=(st and s_ == 0),
                                        stop=(sp and s_ == 1))
                            else:
                                nc.tensor.matmul(
                                    y_ps[:, j, pq0:R],
                                    v_res[:, pk0:pk0 + 2, hh, :],
                                    pm_[:, :, j, 0:R - pq0], start=st, stop=sp,
                                    perf_mode=PM.DoubleRow)
                    if prev is not None:
                        pm_, pk0 = prev
                        av_pair(pm_, pk0, False)
                    prev = (axm, k0)
                pm_, pk0 = prev
                av_pair(pm_, pk0, True)
                for j, hh in enumerate(heads):
                    rc = dnpool.tile([68, R], F32, name="rc", tag="rc")
                    nc.vector.reciprocal(rc[64:65, :], y_ps[64:65, j, :])
                    bc_ps = bcpsum.tile([64, R], F32, name="bc_ps",
                                        tag="bc_ps")
                    nc.tensor.matmul(bc_ps[:], ones65[64:65, :], rc[64:65, :])
                    bc_sb = dnpool.tile([64, R], F32, name="bc_sb",
                                        tag="bc_sb")
                    nc.scalar.copy(bc_sb[:], bc_ps[:])
                    # num/den carries exactly 32*y -> fp8 directly
                    nc.vector.tensor_mul(yT8[:, hh, :], y_ps[0:64, j, :],
                                         bc_sb[:])

            wopsum = apsum
            for tt in range(NT):
                for oh in range(2):
                    pso = wopsum.tile([128, 384], F32, name="pso", tag="pso")
                    if "wo" in NODR:
                        for hh in range(H):
                            nc.tensor.matmul(pso[:],
                                             yT8[:, hh, ts(tt, 128)],
                                             wo_sb[:, hh // 2, hh % 2,
                                                   ts(oh, 384)],
                                             start=(hh == 0),
                                             stop=(hh == H - 1))
                    else:
                        for hp in range(NCT):
                            nc.tensor.matmul(pso[:],
                                             yT8[:, 2 * hp:2 * hp + 2,
                                                 ts(tt, 128)],
                                             wo_sb[:, hp, :, ts(oh, 384)],
                                             start=(hp == 0), stop=(hp == 5),
                                             perf_mode=PM.DoubleRow)
                    att = dnpool.tile([128, 384], BF16, name="att", tag="att")
                    nc.scalar.activation(att[:], pso[:], AF.Copy, scale=DQ_WO)
                    nc.vector.tensor_add(x2_sb[:, tt, ts(oh, 384)], att[:],
                                         x_sb[:, tt, ts(oh, 384)])

        if PHASE.startswith("attn"):
            nc.sync.dma_start(io["out"][:], x2_sb[:])
            apx.close()
            return

        # ---------------- MLP phase ----------------
        apx.close()
        with (
            tc.tile_pool(name="bpool", bufs=1) as bpool,
            tc.tile_pool(name="btpsum", bufs=2, space="PSUM") as btpsum,
            tc.tile_pool(name="g1pool", bufs=4) as g1pool,
        ):
            wpj_sb = bpool.tile([128, 12, 2, C], FP8)
            nc.scalar.dma_start(wpj_sb[:], io["wpjp"][:])

            # ln2 -> h2T (bf16, carries 8*h2)
            h2_sb = bpool.tile([128, NT, C], BF16)
            for tt in range(NT):
                mv, rs = _ln_stats(nc, lnpool, x2_sb[:, tt, :], eps_sb)
                nc.vector.tensor_scalar(
                    out=h2_sb[:, tt, :], in0=x2_sb[:, tt, :],
                    scalar1=mv[:, 0:1], scalar2=rs,
                    op0=mybir.AluOpType.subtract, op1=mybir.AluOpType.mult)
            h2T_sb = bpool.tile([128, NCT, R], BF16)
            for tt in range(NT):
                for ct in range(NCT):
                    pst2 = btpsum.tile([128, 128], BF16, name="pst2",
                                       tag="pst2")
                    nc.tensor.transpose(pst2[:], h2_sb[:, tt, ts(ct, 128)],
                                        id128[:])
                    nc.vector.tensor_copy(h2T_sb[:, ct, ts(tt, 128)], pst2[:])

            # fT (fp8, carries 16*f); h2T carries 8x so dequant is S_F/8
            fT_sb = bpool.tile([128, NJT, R], FP8)
            with tc.tile_pool(name="fpsum", bufs=2, space="PSUM") as fpsum:
                for jt in range(NJT):
                    psf = fpsum.tile([128, R], F32, name="psf", tag="psf")
                    for ct in range(NCT):
                        nc.tensor.matmul(psf[:], wfc_sb[:, ct, ts(jt, 128)],
                                         h2T_sb[:, ct, :], start=(ct == 0),
                                         stop=(ct == 5))
                    nc.scalar.activation(fT_sb[:, jt, :], psf[:], AF.Copy,
                                         scale=S_F / 8.0)

            # g1 = f @ Wsw, g2 = f @ Vsw, fp8 DoubleRow over 12 jt-pairs.
            # Pass (wsw, ph0) uses the SBUF-prefetched wsw0_sb; the other
            # three passes stream chunks round-robin from 3 engine queues.
            g1s_sb = bpool.tile([128, NT, C4], BF16)
            gr_sb = bpool.tile([128, NT, C4], BF16)
            gctx = ExitStack()
            gpsum = gctx.enter_context(
                tc.tile_pool(name="gpsum", bufs=1, space="PSUM"))
            dmaq = [nc.sync, nc.gpsimd, nc.scalar]
            qi = 0
            for wname, warr, ph in (("wswp", "sw", 0), ("wswp", "sw", 1),
                                    ("vswp", "vs", 0), ("vswp", "vs", 1)):
                acc = {}
                for tt in range(NT):
                    for oc in range(3):
                        acc[(tt, oc)] = gpsum.tile(
                            [128, 512], F32, name=f"g{tt}{oc}",
                            tag=f"g{tt}{oc}")
                for jp in range(12):
                    if wname == "wswp" and ph == 0 and jp < NPRE:
                        wch = wsw0_sb[:, jp]
                    else:
                        wcht = wswpool.tile([128, 2, 1536], FP8, name="wch",
                                            tag="wch")
                        dmaq[qi % 3].dma_start(wcht[:], io[wname][ph, jp])
                        qi += 1
                        wch = wcht[:]
                    for tt in range(NT):
                        for oc in range(3):
                            nc.tensor.matmul(
                                acc[(tt, oc)][:],
                                fT_sb[:, 2 * jp:2 * jp + 2, ts(tt, 128)],
                                wch[:, :, ts(oc, 512)],
                                start=(jp == 0), stop=(jp == 11),
                                perf_mode=PM.DoubleRow)
                for tt in range(NT):
                    for oc in range(3):
                        off = (0 if warr == "sw" else 0) + ph * 1536 + oc * 512
                        if warr == "sw":
                            sg = g1pool.tile([128, 512], BF16, name="sgt",
                                             tag="sgt")
                            nc.scalar.activation(sg[:], acc[(tt, oc)][:],
                                                 AF.Sigmoid,
                                                 scale=1.0 / S_ACC)
                            nc.vector.tensor_mul(
                                g1s_sb[:, tt, off:off + 512],
                                acc[(tt, oc)][:], sg[:])
                        else:
                            nc.vector.tensor_mul(
                                gr_sb[:, tt, off:off + 512],
                                acc[(tt, oc)][:],
                                g1s_sb[:, tt, off:off + 512])

            gctx.close()
            # transpose g rows -> gT (fp8, carries 64*g)
            gT8_sb = bpool.tile([128, NJT, R], FP8)
            for tt in range(NT):
                for k in range(NJT):
                    pst3 = btpsum.tile([128, 128], BF16, name="pst3",
                                       tag="pst2")
                    nc.tensor.transpose(pst3[:], gr_sb[:, tt, ts(k, 128)],
                                        id128[:])
                    nc.vector.tensor_scalar(
                        out=gT8_sb[:, k, ts(tt, 128)], in0=pst3[:],
                        scalar1=DQ_GT, scalar2=None,
                        op0=mybir.AluOpType.mult)

            out_sb = bpool.tile([128, NT, C], F32)
            with tc.tile_pool(name="ppsum", bufs=2, space="PSUM") as ppsum:
                for tt in range(NT):
                    for oh in range(2):
                        psp = ppsum.tile([128, 384], F32, name="psp",
                                         tag="psp")
                        for jp in range(12):
                            if "proj" in NODR:
                                for s_ in range(2):
                                    nc.tensor.matmul(
                                        psp[:],
                                        gT8_sb[:, 2 * jp + s_, ts(tt, 128)],
                                        wpj_sb[:, jp, s_, ts(oh, 384)],
                                        start=(jp == 0 and s_ == 0),
                                        stop=(jp == 11 and s_ == 1))
                            else:
                                nc.tensor.matmul(psp[:],
                                                 gT8_sb[:, 2 * jp:2 * jp + 2,
                                                        ts(tt, 128)],
                                                 wpj_sb[:, jp, :, ts(oh, 384)],
                                                 start=(jp == 0),
                                                 stop=(jp == 11),
                                                 perf_mode=PM.DoubleRow)
                        prj = g1pool.tile([128, 384], BF16, name="prj",
                                          tag="prj")
                        nc.scalar.activation(prj[:], psp[:], AF.Copy,
                                             scale=DQ_P)
                        nc.vector.tensor_add(out_sb[:, tt, ts(oh, 384)],
                                             prj[:],
                                             x2_sb[:, tt, ts(oh, 384)])
            nc.sync.dma_start(io["out"][:], out_sb[:])


def build_nc():
    nc = bacc.Bacc("TRN2", target_bir_lowering=False, debug=False,
                   num_devices=NCORES)
    io = {}

    def inp(name, shape, dtype):
        io[name] = nc.dram_tensor(name, shape, dtype,
                                  kind="ExternalInput").ap()

    inp("xp", [128, NT, C], F32)
    inp("xfull", [128, T // 128, C], BF16)
    inp("maskp", [128, NKV, 2 * R], FP8)
    inp("wqp", [128, NCT, C], FP8)
    inp("wkp", [128, NCT, C], FP8)
    inp("wvp", [128, NCT, C], FP8)
    inp("wop", [64, NCT, 2, C], FP8)
    inp("wfcp", [128, NCT, C4], BF16)
    inp("wswp", [2, 12, 128, 2, 1536], FP8)
    inp("vswp", [2, 12, 128, 2, 1536], FP8)
    inp("wpjp", [128, 12, 2, C], FP8)
    io["out"] = nc.dram_tensor("out", [128, NT, C], F32,
                               kind="ExternalOutput").ap()

    with tile.TileContext(nc) as tc:
        _body(tc, io)
    nc.compile()
    return nc


def _arr_pct(w, p=128):
    """(a*p, b) row-major -> (p, a, b) contiguous."""
    a = w.shape[0] // p
    return np.ascontiguousarray(w.reshape(a, p, w.shape[1]).transpose(1, 0, 2))


def _arr_sw(w):
    """(3072, 3072) -> (2, 12, 128, 2, 1536): [pass, jt-pair, p, sub, o]."""
    r = w.reshape(12, 2, 128, 2, 1536).transpose(3, 0, 2, 1, 4)
    return np.ascontiguousarray(r)


def _f8(w, scale):
    f8 = ml_dtypes.float8_e4m3
    return np.clip(np.asarray(w, np.float32) * scale, -240.0, 240.0).astype(f8)


def host_prep(inputs):
    """Cast/scale/transpose weights on host into device-ready layouts."""
    bf16 = ml_dtypes.bfloat16
    f32 = np.float32
    x = np.asarray(inputs["x"], f32)
    Wqkv = np.asarray(inputs["Wqkv"], f32)
    WoT = np.asarray(inputs["Wo"], f32).T          # (768 in, 768 out)
    WpjT = np.asarray(inputs["Wproj"], f32).T      # (3072 in, 768 out)
    shared = {
        "xfull": np.ascontiguousarray(
            x.reshape(T // 128, 128, C).transpose(1, 0, 2)).astype(bf16),
        "wqp": _f8(_arr_pct(Wqkv[0:C].T.astype(f32)), S_W),
        "wkp": _f8(_arr_pct(Wqkv[C:2 * C].T.astype(f32)), S_W),
        "wvp": _f8(_arr_pct(Wqkv[2 * C:3 * C].T.astype(f32)), S_W),
        "wop": _f8(np.ascontiguousarray(
            WoT.reshape(NCT, 2, 64, C).transpose(2, 0, 1, 3)), S_W),
        "wfcp": _arr_pct(np.asarray(inputs["Wfc"], f32).T.astype(bf16)),
        "wswp": _f8(_arr_sw(np.asarray(inputs["Wsw"], f32)), S_W),
        "vswp": _f8(_arr_sw(np.asarray(inputs["Vsw"], f32)), S_W),
        "wpjp": _f8(np.ascontiguousarray(
            WpjT.reshape(12, 2, 128, C).transpose(2, 0, 1, 3)), S_W),
    }
    kv = np.arange(T, dtype=np.int64)
    in_maps = []
    for i in range(NCORES):
        # balanced causal: core i owns q-tiles {i, 8+i}
        ta, tb = i, 8 + i
        row = np.concatenate([128 * ta + np.arange(128, dtype=np.int64),
                              128 * tb + np.arange(128, dtype=np.int64)])
        mask = (kv[:, None] <= row[None, :]).astype(f32)  # (T, 256)
        mp = mask.reshape(NKV, 128, R).transpose(1, 0, 2)      # (128, NKV, R)
        mp4 = np.broadcast_to(mp[:, :, None, :], (128, NKV, 2, R))
        xrows = np.stack([x[128 * ta:128 * ta + 128],
                          x[128 * tb:128 * tb + 128]])          # (2, 128, C)
        in_maps.append({
            "xp": np.ascontiguousarray(xrows.transpose(1, 0, 2)),
            "maskp": np.ascontiguousarray(
                mp4.reshape(128, NKV, 2 * R)).astype(ml_dtypes.float8_e4m3),
            **shared,
        })
    return in_maps


def unshard_out(res_list):
    out = np.empty((T, C), np.float32)
    for i in range(NCORES):
        o = np.asarray(res_list[i]["out"]).reshape(128, NT, C)
        out[128 * i:128 * i + 128] = o[:, 0, :]
        out[128 * (8 + i):128 * (8 + i) + 128] = o[:, 1, :]
    return out


_NC = None


def kernel(**inputs):
    global _NC
    if _NC is None:
        _NC = build_nc()
    in_maps = host_prep(inputs)
    from concourse.bass_interp import get_hw_module
    old_m = _NC.m
    _NC.m = get_hw_module(_NC.m)
    try:
        res = bass_utils.run_bass_kernel_spmd(
            _NC, in_maps, core_ids=list(range(NCORES)))
    finally:
        _NC.m = old_m
    return unshard_out(res.results)


if __name__ == "__main__":
    nc = build_nc()
    print("build + compile OK;",
          sum(len(b.instructions) for f in nc.m.functions for b in f.blocks),
          "instructions")
